# revision 1
# baseline (speedup 1.0000x reference)
"""Bass/Tile kernel for one transformer decoder layer, data-parallel over batch.

Per-core work (one batch element): LN1 -> QKV -> causal attention with
query-axis softmax -> LN2 -> FFN(gelu).

Key math note: the reference softmaxes over the QUERY axis (axis=2), i.e. each
key-column k is normalized over queries q >= k. Therefore
  out[q,d] = sum_k exp(s[q,k]) * (V[k,d] / Z[k]),   Z[k] = sum_{q>=k} exp(s[q,k])
so the 1/Z folds into V's rows and no [T,T] division is needed.
We compute ST = S^T in [k, q] layout (ST = K @ Q^T / sqrt(E)) so that
Z is a free-axis reduction and the AV matmul needs no transposes.

Structure per core:
  x [T,E] --LN1--> xn --PE transpose--> xnT [E,T] fp32r (g/b fused on evict)
  V = xn @ Wv (token-major), then per head pair tt:
      QT/KT rows for pair tt (rotating slots)
      ST blocks -> additive causal mask on PSUM -> one wide exp -> E1T
      previous pair: Z = rowsum(E1T), V' = V * (1/Z), O += E1T^T @ V'
      (O accumulates straight into a [q, (qi,h,d)] concat tile)
  LN2(concat) -> anT;  FFN in two ff-halves (weights fetched once),
  gelu+bias fused;  out written transposed [E,T] (host transposes back).
All big matmuls run in fp32r (fp32 storage, full-rate PE mode); attention
probabilities/V are bf16.
"""

from contextlib import ExitStack

import numpy as np

import concourse.bass as bass
import concourse.tile as tile
import concourse.bacc as bacc
from concourse import mybir

F32 = mybir.dt.float32
F32R = mybir.dt.float32r
BF16 = mybir.dt.bfloat16
AF = mybir.ActivationFunctionType
ALU = mybir.AluOpType
AX = mybir.AxisListType

P = 128
T = 1024
E = 1024
H = 16
HD = 64
FF = 4096
NT = T // P
NE = E // P
NF = FF // P
SCALE = 1.0 / 32.0  # 1/sqrt(E)
EPS = 1e-5

ATT_DT = BF16

# packed const layout (columns in the "consts" dram tensor)
_CONST_COLS = {
    "bqt": (0, NE), "bkt": (NE, NE), "b2t": (2 * NE, NE), "g1t": (3 * NE, NE),
    "be1t": (4 * NE, NE), "g2t": (5 * NE, NE), "be2t": (6 * NE, NE),
    "b1t": (7 * NE, NF), "ident": (7 * NE + NF, P), "maskd": (7 * NE + NF + P, P),
    "bvb": (7 * NE + NF + 2 * P, E),
}
_CONST_W = 7 * NE + NF + 2 * P + E


def build_decoder(debug=False, att_dt=None, reps=1):
    global ATT_DT
    if att_dt is not None:
        ATT_DT = att_dt
    nc = bacc.Bacc(None, target_bir_lowering=False, debug=debug)

    # ---------------- DRAM I/O ----------------
    x_d = nc.dram_tensor("x", (T, E), F32, kind="ExternalInput")
    wq_d = nc.dram_tensor("wq", (NE, P, E), F32R, kind="ExternalInput")
    wk_d = nc.dram_tensor("wk", (NE, P, E), F32R, kind="ExternalInput")
    wv_d = nc.dram_tensor("wv", (E, E), F32R, kind="ExternalInput")
    w1_d = nc.dram_tensor("w1r", (NF, P, E), F32R, kind="ExternalInput")
    w2_d = nc.dram_tensor("w2r", (NE, P, FF), F32R, kind="ExternalInput")
    c_d = nc.dram_tensor("consts", (P, _CONST_W), F32, kind="ExternalInput")
    y_d = nc.dram_tensor("yT", (E, T), F32, kind="ExternalOutput")

    with ExitStack() as es:
        tc = es.enter_context(tile.TileContext(nc))

        const = es.enter_context(tc.tile_pool(name="const", bufs=1, side="left"))
        # one PSUM pool, 8 banks total: 3x [P,1024] (proj/scores/FFN) +
        # 2x [P,512] shared by AV accumulation and LN transposes (never overlap)
        psq = es.enter_context(tc.tile_pool(name="psq", bufs=1, space="PSUM"))

        cz = const.tile([P, _CONST_W], F32)
        nc.sync.dma_start(cz, c_d[:, :])

        def cview(name):
            o, w = _CONST_COLS[name]
            return cz[:, o:o + w]

        bq_t, bk_t, b2_t = cview("bqt"), cview("bkt"), cview("b2t")
        g1_t, be1_t, g2_t, be2_t = (cview("g1t"), cview("be1t"),
                                    cview("g2t"), cview("be2t"))
        b1_t, ident, maskd, bv_b = (cview("b1t"), cview("ident"),
                                    cview("maskd"), cview("bvb"))
        eps_t = const.tile([P, 1], F32)
        nc.vector.memset(eps_t, EPS)
        ident_r = const.tile([P, P], F32R)
        nc.vector.tensor_copy(ident_r, ident)

        def sc_tile(name):
            return psq.tile([P, 2 * 512], F32, tag="sc", bufs=3, name=name)

        def av_tile(name, dt=F32):
            return psq.tile([P, 512], dt, tag="avtr", bufs=2, name=name)

        def layernorm_to_transposed(src_fn, dst_tiles, g_t, b_t, ln_pool):
            """src_fn(ti) -> [P, E] token-major AP.  Writes
            dst_tiles[j][:, ti*P:(ti+1)*P] = norm(src)^T * g + b (feature-major)."""
            for ti in range(NT):
                xsrc = src_fn(ti)
                stats = ln_pool.tile([P, 2, 6], F32, tag="bnstats")
                for sg in range(2):
                    nc.vector.bn_stats(out=stats[:, sg, :],
                                       in_=xsrc[:, sg * 512:(sg + 1) * 512])
                mv = ln_pool.tile([P, 2], F32, tag="bnmv")
                nc.vector.bn_aggr(out=mv, in_=stats)
                nc.scalar.activation(out=mv[:, 1:2], in_=mv[:, 1:2], func=AF.Sqrt,
                                     bias=eps_t)
                nc.vector.reciprocal(mv[:, 1:2], mv[:, 1:2])
                xn = ln_pool.tile([P, E], F32R, tag="xn")
                nc.vector.tensor_scalar(
                    out=xn, in0=xsrc, scalar1=mv[:, 0:1], scalar2=mv[:, 1:2],
                    op0=ALU.subtract, op1=ALU.mult)
                for j in range(NE):
                    ptr = av_tile(f"tr{ti}_{j}", dt=F32R)[:, :P]
                    nc.tensor.transpose(ptr, xn[:, j * P:(j + 1) * P], ident_r)
                    nc.scalar.activation(
                        out=dst_tiles[j][:, ti * P:(ti + 1) * P], in_=ptr,
                        func=AF.Identity, bias=b_t[:, j:j + 1], scale=g_t[:, j:j + 1])

        rep_cm = tc.For_i(0, reps, 1) if reps > 1 else None
        if rep_cm is not None:
            rep_cm.__enter__()

        # =========== Phase 1: LN1 + transpose ===========
        xnT_pool = tc.alloc_tile_pool(name="xnT", bufs=1, side="right")
        xnT = [xnT_pool.tile([P, T], F32R, tag=f"xnT{j}", name=f"xnT{j}")
               for j in range(NE)]
        with tc.tile_pool(name="ph1", bufs=3, side="left") as ph1, \
             tc.tile_pool(name="xin", bufs=NT, side="left") as xin:
            x_tiles = []
            for ti in range(NT):
                xt = xin.tile([P, E], F32, tag="x")
                nc.sync.dma_start(xt, x_d[ti * P:(ti + 1) * P, :])
                x_tiles.append(xt)
            layernorm_to_transposed(lambda ti: x_tiles[ti][:, :], xnT,
                                    g1_t, be1_t, ph1)

        # ====== Phase 2+3: V, then per-pair {Q/K proj -> scores+exp -> AV} ======
        concat_pool = tc.alloc_tile_pool(name="concat", bufs=1, side="left")
        concat = concat_pool.tile([P, NT * E], ATT_DT)
        concat_v = concat.rearrange("p (a h d) -> p a h d", a=NT, h=H)

        vt_pool = tc.alloc_tile_pool(name="vtp", bufs=1, side="left")
        vt = [vt_pool.tile([P, E], ATT_DT, tag=f"vt{i}", name=f"vt{i}")
              for i in range(NT)]

        # --- V projection (token-major) ---
        with tc.tile_pool(name="wv", bufs=NE + 1, side="left") as wvp:
            wtl = []
            for ko in range(NE):
                wtile = wvp.tile([P, E], F32R, tag="w", name=f"wv{ko}")
                nc.sync.dma_start(wtile, wv_d[ko * P:(ko + 1) * P, :])
                wtl.append(wtile)
            for ti in range(NT):
                ps = sc_tile(f"psv{ti}")
                for nh in range(2):
                    for ko in range(NE):
                        nc.tensor.matmul(
                            ps[:, nh * 512:(nh + 1) * 512],
                            lhsT=xnT[ko][:, ti * P:(ti + 1) * P],
                            rhs=wtl[ko][:, nh * 512:(nh + 1) * 512],
                            start=(ko == 0), stop=(ko == NE - 1))
                nc.vector.tensor_add(out=ps, in0=ps, in1=bv_b)
                nc.vector.tensor_copy(out=vt[ti], in_=ps)

        # --- interleaved Q/K projection + attention ---
        qk_pool = tc.alloc_tile_pool(name="qk", bufs=3, side="left")
        wqk_pool = tc.alloc_tile_pool(name="wqk", bufs=6, side="left")
        e1p = tc.alloc_tile_pool(name="e1", bufs=32, side="left")
        vpp = tc.alloc_tile_pool(name="vp", bufs=16, side="left")
        zsp = tc.alloc_tile_pool(name="zs", bufs=4, side="left")

        def emit_qk(tt):
            qtt = qk_pool.tile([P, T], F32R, tag="qt", name=f"qt{tt}")
            ktt = qk_pool.tile([P, T], F32R, tag="kt", name=f"kt{tt}")
            for dst, w_d, b_t in ((qtt, wq_d, bq_t), (ktt, wk_d, bk_t)):
                wsl = wqk_pool.tile([P, NE, P], F32R, tag="wqk")
                nc.sync.dma_start(wsl, w_d[tt].rearrange("p (a b) -> p a b", a=NE))
                ps = sc_tile(f"psqk{tt}")
                for th in range(2):
                    for ko in range(NE):
                        nc.tensor.matmul(
                            ps[:, th * 512:(th + 1) * 512],
                            lhsT=wsl[:, ko, :],
                            rhs=xnT[ko][:, th * 512:(th + 1) * 512],
                            start=(ko == 0), stop=(ko == NE - 1))
                nc.vector.tensor_scalar(
                    out=dst, in0=ps, scalar1=b_t[:, tt:tt + 1], scalar2=None,
                    op0=ALU.add)
            return qtt, ktt

        def emit_scores(tt, qtt, ktt, e1s, zhs):
            """ST blocks + additive mask + one wide exp per (h, ki); even/odd
            heads interleaved so their K=64 matmuls share the PE array.
            Z reductions run eagerly after each exp so the AV stage only
            waits on reciprocal+V'."""
            for ki in range(NT):
                for h in (2 * tt, 2 * tt + 1):
                    po = HD * (h % 2)
                    e1ki = e1p.tile([P, T], ATT_DT, tag="e1t", name=f"e1_{h}_{ki}")
                    e1s[(h, ki)] = e1ki
                    c0 = (ki * P) // 512
                    sps = sc_tile(f"pss{h}_{ki}")
                    for c in range(c0, 2):
                        nc.tensor.matmul(
                            sps[:, c * 512:(c + 1) * 512],
                            lhsT=ktt[po:po + HD, ki * P:(ki + 1) * P],
                            rhs=qtt[po:po + HD, c * 512:(c + 1) * 512],
                            start=True, stop=True)
                    nc.vector.tensor_add(
                        out=sps[:, ki * P:(ki + 1) * P],
                        in0=sps[:, ki * P:(ki + 1) * P], in1=maskd)
                    nc.scalar.activation(
                        out=e1ki[:, ki * P:], in_=sps[:, ki * P:],
                        func=AF.Exp, scale=SCALE)

        def emit_av(hp, e1s, zhs):
            for h in (2 * hp, 2 * hp + 1):
                zh = zsp.tile([P, NT], F32, tag="zh")
                for ki in range(NT):
                    nc.vector.reduce_sum(out=zh[:, ki:ki + 1],
                                         in_=e1s[(h, ki)][:, ki * P:], axis=AX.X)
                rz = zsp.tile([P, NT], F32, tag="rz")
                nc.vector.reciprocal(rz, zh)
                vps = []
                for ki in range(NT):
                    vp_t = vpp.tile([P, HD], ATT_DT, tag="vp")
                    nc.vector.tensor_scalar_mul(
                        vp_t, vt[ki][:, h * HD:(h + 1) * HD], rz[:, ki:ki + 1])
                    vps.append(vp_t)
                po_ps = av_tile(f"psav{h}")
                for qi in range(NT):
                    for ki in range(qi + 1):
                        nc.tensor.matmul(
                            po_ps[:, qi * HD:(qi + 1) * HD],
                            lhsT=e1s[(h, ki)][:, qi * P:(qi + 1) * P],
                            rhs=vps[ki],
                            start=(ki == 0), stop=(ki == qi))
                nc.vector.tensor_copy(
                    out=concat_v[:, :, h, :],
                    in_=po_ps.rearrange("p (a d) -> p a d", a=NT))

        e1s_by_hp = {}
        zhs = {}
        for tt in range(H // 2):
            qtt, ktt = emit_qk(tt)
            e1s_by_hp[tt] = {}
            emit_scores(tt, qtt, ktt, e1s_by_hp[tt], zhs)
            if tt >= 1:
                emit_av(tt - 1, e1s_by_hp.pop(tt - 1), zhs)
        emit_av(H // 2 - 1, e1s_by_hp.pop(H // 2 - 1), zhs)

        zsp.release(); vpp.release(); e1p.release()
        wqk_pool.release(); qk_pool.release()
        vt_pool.release()
        xnT_pool.release()

        # =========== Phase 4: LN2 + transpose ===========
        anT_pool = tc.alloc_tile_pool(name="anT", bufs=1, side="right")
        anT = [anT_pool.tile([P, T], F32R, tag=f"anT{j}", name=f"anT{j}")
               for j in range(NE)]
        with tc.tile_pool(name="ph4", bufs=3, side="left") as ph4:
            layernorm_to_transposed(
                lambda ti: concat[:, ti * E:(ti + 1) * E], anT, g2_t, be2_t, ph4)
        concat_pool.release()

        # =========== Phase 5: FFN (two ff halves; weights fetched once) ===========
        NFH = NF // 2
        out_pool = tc.alloc_tile_pool(name="outT", bufs=1, side="right")
        outT = [out_pool.tile([P, T], F32, tag=f"o{j}", name=f"o{j}")
                for j in range(NE)]
        with tc.tile_pool(name="w1s", bufs=4, side="left") as w1s, \
             tc.tile_pool(name="w2s", bufs=3, side="left") as w2s, \
             tc.tile_pool(name="hid", bufs=NF // 2 + 1, side="left") as hidp:
            for ffh in range(2):
                hid = []
                for fl in range(NFH):
                    fo = ffh * NFH + fl
                    w1t = w1s.tile([P, NE, P], F32R, tag="w1")
                    nc.sync.dma_start(w1t, w1_d[fo].rearrange("p (a b) -> p a b", a=NE))
                    hid_fo = hidp.tile([P, T], F32R, tag="hid", name=f"hid{fo}")
                    ps = sc_tile(f"psf{fo}")
                    for th in range(2):
                        for ko in range(NE):
                            nc.tensor.matmul(
                                ps[:, th * 512:(th + 1) * 512],
                                lhsT=w1t[:, ko, :],
                                rhs=anT[ko][:, th * 512:(th + 1) * 512],
                                start=(ko == 0), stop=(ko == NE - 1))
                    nc.scalar.activation(
                        out=hid_fo, in_=ps, func=AF.Gelu, bias=b1_t[:, fo:fo + 1])
                    hid.append(hid_fo)
                for eo in range(NE):
                    w2t = w2s.tile([P, NFH, P], F32R, tag="w2")
                    nc.sync.dma_start(
                        w2t, w2_d[eo, :, ffh * NFH * P:(ffh + 1) * NFH * P]
                        .rearrange("p (a b) -> p a b", a=NFH))
                    ps = sc_tile(f"pso{ffh}_{eo}")
                    for th in range(2):
                        for kl in range(NFH):
                            nc.tensor.matmul(
                                ps[:, th * 512:(th + 1) * 512],
                                lhsT=w2t[:, kl, :],
                                rhs=hid[kl][:, th * 512:(th + 1) * 512],
                                start=(kl == 0), stop=(kl == NFH - 1))
                    if ffh == 0:
                        nc.scalar.activation(
                            out=outT[eo], in_=ps,
                            func=AF.Identity, bias=b2_t[:, eo:eo + 1])
                    else:
                        nc.vector.tensor_add(out=outT[eo], in0=ps, in1=outT[eo])
        for eo in range(NE):
            nc.sync.dma_start(y_d[eo * P:(eo + 1) * P, :], outT[eo])
        out_pool.release()
        anT_pool.release()
        if rep_cm is not None:
            rep_cm.__exit__(None, None, None)

    nc.compile()
    return nc


def host_inputs(core_x, Wq, bq, Wk, bk, Wv, bv, W1, b1, W2, b2, g1, be1, g2, be2):
    """Build the per-core in_map from full weights + this core's x shard [T, E]."""
    f = np.float32

    consts = np.zeros((P, _CONST_W), f)

    def put(name, arr):
        o, w = _CONST_COLS[name]
        consts[:, o:o + w] = arr

    put("bqt", np.asarray(bq, f).reshape(NE, P).T)
    put("bkt", np.asarray(bk, f).reshape(NE, P).T)
    put("b2t", np.asarray(b2, f).reshape(NE, P).T)
    put("g1t", np.asarray(g1, f).reshape(NE, P).T)
    put("be1t", np.asarray(be1, f).reshape(NE, P).T)
    put("g2t", np.asarray(g2, f).reshape(NE, P).T)
    put("be2t", np.asarray(be2, f).reshape(NE, P).T)
    put("b1t", np.asarray(b1, f).reshape(NF, P).T)
    put("ident", np.eye(P, dtype=f))
    put("maskd", np.where(np.triu(np.ones((P, P), bool)), 0.0, -6000.0).astype(f))
    put("bvb", np.broadcast_to(np.asarray(bv, f), (P, E)))

    return {
        "x": np.ascontiguousarray(core_x, f),
        "wq": np.ascontiguousarray(
            np.asarray(Wq, f).reshape(NE, P, NE, P).transpose(2, 1, 0, 3)
            .reshape(NE, P, E)),
        "wk": np.ascontiguousarray(
            np.asarray(Wk, f).reshape(NE, P, NE, P).transpose(2, 1, 0, 3)
            .reshape(NE, P, E)),
        "wv": np.ascontiguousarray(Wv, f),
        "w1r": np.ascontiguousarray(
            np.asarray(W1, f).reshape(NE, P, NF, P).transpose(2, 1, 0, 3)
            .reshape(NF, P, E)),
        "w2r": np.ascontiguousarray(
            np.asarray(W2, f).reshape(NF, P, NE, P).transpose(2, 1, 0, 3)
            .reshape(NE, P, FF)),
        "consts": consts,
    }



# ======================================================================
# Harness entry point: full-input kernel with internal batch sharding
# ======================================================================

_NC_CACHE = {}


def _get_nc():
    key = ("bf16" if ATT_DT == BF16 else "f32")
    if key not in _NC_CACHE:
        _NC_CACHE[key] = build_decoder()
    return _NC_CACHE[key]


def kernel(x, Wq, bq, Wk, bk, Wv, bv, W1, b1, W2, b2, g1, be1, g2, be2):
    """Full-input entry: x [8, 1024, 1024]; returns [8, 1024, 1024] float32.

    Shards batch across the 8 NeuronCores (one element per core), runs the
    Bass decoder kernel SPMD, and gathers/transposes the per-core outputs.
    """
    from concourse.bass_utils import run_bass_kernel_spmd

    x = np.asarray(x, np.float32)
    B = x.shape[0]
    nc = _get_nc()
    args = tuple(np.asarray(a, np.float32) for a in
                 (Wq, bq, Wk, bk, Wv, bv, W1, b1, W2, b2, g1, be1, g2, be2))
    in_maps = [host_inputs(x[c], *args) for c in range(B)]
    res = run_bass_kernel_spmd(nc, in_maps, core_ids=list(range(B)))
    out = np.stack([np.asarray(r["yT"]).T for r in res.results], axis=0)
    return np.ascontiguousarray(out, np.float32)



# revision 6
# speedup vs baseline: 1.1594x; 1.1594x over previous
"""Bass/Tile kernel for one transformer decoder layer, data-parallel over batch.

Per-core work (one batch element): LN1 -> QKV -> causal attention with
query-axis softmax -> LN2 -> FFN(gelu).

Key math note: the reference softmaxes over the QUERY axis (axis=2), i.e. each
key-column k is normalized over queries q >= k. Therefore
  out[q,d] = sum_k exp(s[q,k]) * (V[k,d] / Z[k]),   Z[k] = sum_{q>=k} exp(s[q,k])
so the 1/Z folds into V's rows and no [T,T] division is needed.
We compute ST = S^T in [k, q] layout (ST = K @ Q^T / sqrt(E)) so that
Z is a free-axis reduction and the AV matmul needs no transposes.

All large GEMMs (Q/K/V projections and both FFN layers) run in fp8e4
DoubleRow mode with residual compensation: weights are host-split into
W ~ (W8 + R8)/SW with both parts e4m3; activations are split on-chip into
X8 + RX8 by quantize + subtract. The product is computed as
  X8@W8 + X8@R8 + RX8@W8   (the RX8@R8 term is ~eps^2 and dropped)
with all three terms accumulated in one PSUM group, so dequantization is a
single 1/SW at eviction. DoubleRow packs two 128-row K-tiles per matmul at
0.5 cycles/row, so the 3-term compensated GEMM costs 0.75x an fp32r GEMM
while keeping ~fp32 accuracy (residuals capture the quantization error).

Attention scores/AV stay in fp32r/bf16. Z row-sums are fused into the exp
activation via accum_out; the causal mask add runs on the (otherwise idle)
Pool engine.

Structure per core:
  x [T,E] --LN1--> xn --PE transpose--> xnT8/xnTr [E,T] fp8 pair
  V = xn @ Wv (token-major), then per head pair tt:
      QT/KT rows for pair tt (rotating slots)
      ST blocks -> additive causal mask on PSUM (Pool) -> one wide exp with
      fused Z accumulation -> E1T
      previous pair: V' = V * (1/Z), O += E1T^T @ V'
  LN2(concat) -> anT8/anTr;  FFN in two ff-halves (weights fetched once),
  gelu+bias fused, hid split to fp8 pair;  out written transposed [E,T].
"""

from contextlib import ExitStack

import numpy as np

import concourse.bass as bass
import concourse.tile as tile
import concourse.bacc as bacc
from concourse import mybir

F32 = mybir.dt.float32
F32R = mybir.dt.float32r
BF16 = mybir.dt.bfloat16
FP8 = mybir.dt.float8e4
AF = mybir.ActivationFunctionType
ALU = mybir.AluOpType
AX = mybir.AxisListType
DR = mybir.MatmulPerfMode.DoubleRow

P = 128
T = 1024
E = 1024
H = 16
HD = 64
FF = 4096
NT = T // P
NE = E // P
NF = FF // P
SCALE = 1.0 / 32.0  # 1/sqrt(E)
EPS = 1e-5
SW = 32.0  # host-side fp8 weight scale; divided out at PSUM eviction
RSW = 1.0 / SW

ATT_DT = BF16

# packed const layout (columns in the "consts" dram tensor)
_CONST_COLS = {
    "bqt": (0, NE), "bkt": (NE, NE), "b2t": (2 * NE, NE), "g1t": (3 * NE, NE),
    "be1t": (4 * NE, NE), "g2t": (5 * NE, NE), "be2t": (6 * NE, NE),
    "b1t": (7 * NE, NF), "ident": (7 * NE + NF, P), "maskd": (7 * NE + NF + P, P),
    "bvb": (7 * NE + NF + 2 * P, E),
}
_CONST_W = 7 * NE + NF + 2 * P + E


def build_decoder(debug=False, att_dt=None, reps=1):
    global ATT_DT
    if att_dt is not None:
        ATT_DT = att_dt
    nc = bacc.Bacc(None, target_bir_lowering=False, debug=debug)

    # ---------------- DRAM I/O ----------------
    x_d = nc.dram_tensor("x", (T, E), F32, kind="ExternalInput")
    wq_d = nc.dram_tensor("wq8", (2, NE, P, E), FP8, kind="ExternalInput")
    wk_d = nc.dram_tensor("wk8", (2, NE, P, E), FP8, kind="ExternalInput")
    wv_d = nc.dram_tensor("wv8", (2, NE // 2, P, 2, E), FP8, kind="ExternalInput")
    w1_d = nc.dram_tensor("w18", (2, NF, P, E), FP8, kind="ExternalInput")
    w2_d = nc.dram_tensor("w28", (2, NE, P, FF), FP8, kind="ExternalInput")
    c_d = nc.dram_tensor("consts", (P, _CONST_W), F32, kind="ExternalInput")
    y_d = nc.dram_tensor("yT", (E, T), F32, kind="ExternalOutput")

    with ExitStack() as es:
        tc = es.enter_context(tile.TileContext(nc))

        const = es.enter_context(tc.tile_pool(name="const", bufs=1, side="left"))
        # one PSUM pool, 8 banks total: 3x [P,1024] (proj/scores/FFN) +
        # 2x [P,512] shared by AV accumulation and LN transposes (never overlap)
        psq = es.enter_context(tc.tile_pool(name="psq", bufs=1, space="PSUM"))

        cz = const.tile([P, _CONST_W], F32)
        nc.sync.dma_start(cz, c_d[:, :])

        def cview(name):
            o, w = _CONST_COLS[name]
            return cz[:, o:o + w]

        bq_t, bk_t, b2_t = cview("bqt"), cview("bkt"), cview("b2t")
        g1_t, be1_t, g2_t, be2_t = (cview("g1t"), cview("be1t"),
                                    cview("g2t"), cview("be2t"))
        b1_t, ident, maskd, bv_b = (cview("b1t"), cview("ident"),
                                    cview("maskd"), cview("bvb"))
        eps_t = const.tile([P, 1], F32)
        nc.vector.memset(eps_t, EPS)
        ident_r = const.tile([P, P], F32R)
        nc.vector.tensor_copy(ident_r, ident)

        def sc_tile(name):
            return psq.tile([P, 2 * 512], F32, tag="sc", bufs=3, name=name)

        def av_tile(name, dt=F32):
            return psq.tile([P, 512], dt, tag="avtr", bufs=2, name=name)

        def layernorm_to_transposed(src_fn, dst_q, dst_r, g_t, b_t, ln_pool):
            """src_fn(ti) -> [P, E] token-major AP.  Writes the fp8 pair
            dst_q/dst_r [P, NE, T] feature-major: q = fp8(norm^T*g+b),
            r = fp8(full - q)."""
            for ti in range(NT):
                xsrc = src_fn(ti)
                stats = ln_pool.tile([P, 2, 6], F32, tag="bnstats")
                for sg in range(2):
                    nc.vector.bn_stats(out=stats[:, sg, :],
                                       in_=xsrc[:, sg * 512:(sg + 1) * 512])
                mv = ln_pool.tile([P, 2], F32, tag="bnmv")
                nc.vector.bn_aggr(out=mv, in_=stats)
                nc.scalar.activation(out=mv[:, 1:2], in_=mv[:, 1:2], func=AF.Sqrt,
                                     bias=eps_t)
                nc.vector.reciprocal(mv[:, 1:2], mv[:, 1:2])
                xn = ln_pool.tile([P, E], F32R, tag="xn")
                nc.vector.tensor_scalar(
                    out=xn, in0=xsrc, scalar1=mv[:, 0:1], scalar2=mv[:, 1:2],
                    op0=ALU.subtract, op1=ALU.mult)
                for j in range(NE):
                    ptr = av_tile(f"tr{ti}_{j}", dt=F32R)[:, :P]
                    nc.tensor.transpose(ptr, xn[:, j * P:(j + 1) * P], ident_r)
                    # full-precision normalized block (g,b applied), then split
                    # into fp8 main + fp8 residual
                    tmp = ln_pool.tile([P, P], F32, tag="lntmp")
                    nc.scalar.activation(
                        out=tmp, in_=ptr,
                        func=AF.Identity, bias=b_t[:, j:j + 1], scale=g_t[:, j:j + 1])
                    qv = dst_q[:, j, ti * P:(ti + 1) * P]
                    nc.vector.tensor_copy(out=qv, in_=tmp)
                    nc.vector.tensor_tensor(
                        out=dst_r[:, j, ti * P:(ti + 1) * P],
                        in0=tmp, in1=qv, op=ALU.subtract)

        def mm3(ps_slice, wq_tile, wr_tile, xq_ap, xr_ap, nk):
            """3-term compensated fp8 DoubleRow GEMM into one PSUM group.
            wq_tile/wr_tile: [P, nk, M] fp8; xq_ap/xr_ap: [P, nk, N] fp8."""
            terms = ((wq_tile, xq_ap), (wr_tile, xq_ap), (wq_tile, xr_ap))
            nkh = nk // 2
            n_inst = 3 * nkh
            i = 0
            for wt, xt in terms:
                for kp in range(nkh):
                    nc.tensor.matmul(
                        ps_slice,
                        lhsT=wt[:, 2 * kp:2 * kp + 2, :],
                        rhs=xt[:, 2 * kp:2 * kp + 2, :],
                        start=(i == 0), stop=(i == n_inst - 1), perf_mode=DR)
                    i += 1

        rep_cm = tc.For_i(0, reps, 1) if reps > 1 else None
        if rep_cm is not None:
            rep_cm.__enter__()

        # =========== Phase 1: LN1 + transpose (fp8 pair) ===========
        xnT_pool = tc.alloc_tile_pool(name="xnT", bufs=1, side="right")
        xnT8 = xnT_pool.tile([P, NE, T], FP8, tag="xnT8", name="xnT8")
        xnTr = xnT_pool.tile([P, NE, T], FP8, tag="xnTr", name="xnTr")
        with tc.tile_pool(name="ph1", bufs=3, side="left") as ph1, \
             tc.tile_pool(name="xin", bufs=NT, side="left") as xin:
            x_tiles = []
            for ti in range(NT):
                xt = xin.tile([P, E], F32, tag="x")
                nc.sync.dma_start(xt, x_d[ti * P:(ti + 1) * P, :])
                x_tiles.append(xt)
            layernorm_to_transposed(lambda ti: x_tiles[ti][:, :], xnT8, xnTr,
                                    g1_t, be1_t, ph1)

        # ====== Phase 2+3: V, then per-pair {Q/K proj -> scores+exp -> AV} ======
        # w1s is created early so it sits below concat on the left pool stack
        # (LIFO release order); its DMAs are issued at the start of LN2.
        NFH = NF // 2
        w1s = tc.alloc_tile_pool(name="w1s", bufs=NFH + 1, side="left")
        concat_pool = tc.alloc_tile_pool(name="concat", bufs=1, side="left")
        concat = concat_pool.tile([P, NT * E], ATT_DT)
        concat_v = concat.rearrange("p (a h d) -> p a h d", a=NT, h=H)

        vt_pool = tc.alloc_tile_pool(name="vtp", bufs=1, side="left")
        vt = [vt_pool.tile([P, E], ATT_DT, tag=f"vt{i}", name=f"vt{i}")
              for i in range(NT)]

        # --- V projection (token-major): xn stationary, Wv moving ---
        with tc.tile_pool(name="wv", bufs=NE + 2, side="left") as wvp:
            wtl = []
            for m in range(2):  # main, residual
                for ko in range(NE // 2):
                    wtile = wvp.tile([P, 2, E], FP8, tag="w", name=f"wv{m}_{ko}")
                    nc.sync.dma_start(wtile, wv_d[m, ko])
                    wtl.append(wtile)
            for ti in range(NT):
                ps = sc_tile(f"psv{ti}")
                for nh in range(2):
                    pss = ps[:, nh * 512:(nh + 1) * 512]
                    i = 0
                    for xs, woff in ((xnT8, 0), (xnT8, NE // 2), (xnTr, 0)):
                        for kp in range(NE // 2):
                            nc.tensor.matmul(
                                pss,
                                lhsT=xs[:, 2 * kp:2 * kp + 2, ti * P:(ti + 1) * P],
                                rhs=wtl[woff + kp][:, :, nh * 512:(nh + 1) * 512],
                                start=(i == 0), stop=(i == 3 * (NE // 2) - 1),
                                perf_mode=DR)
                            i += 1
                vtmp = wvp.tile([P, E], F32, tag="vtmp", bufs=2)
                nc.vector.tensor_scalar(
                    out=vtmp, in0=ps, scalar1=RSW, scalar2=None, op0=ALU.mult)
                nc.vector.tensor_add(out=vt[ti], in0=vtmp, in1=bv_b)

        # --- interleaved Q/K projection + attention ---
        qk_pool = tc.alloc_tile_pool(name="qk", bufs=2, side="left")
        wqk_pool = tc.alloc_tile_pool(name="wqk", bufs=6, side="left")
        e1p = tc.alloc_tile_pool(name="e1", bufs=32, side="left")
        vpp = tc.alloc_tile_pool(name="vp", bufs=16, side="left")
        zsp = tc.alloc_tile_pool(name="zs", bufs=8, side="left")

        def emit_qk(tt):
            qtt = qk_pool.tile([P, T], F32R, tag="qt", name=f"qt{tt}")
            ktt = qk_pool.tile([P, T], F32R, tag="kt", name=f"kt{tt}")
            for dst, w_d, b_t in ((qtt, wq_d, bq_t), (ktt, wk_d, bk_t)):
                wsl = wqk_pool.tile([P, NE, P], FP8, tag="wqk")
                wsr = wqk_pool.tile([P, NE, P], FP8, tag="wqkr")
                nc.sync.dma_start(wsl, w_d[0, tt].rearrange("p (a b) -> p a b", a=NE))
                nc.sync.dma_start(wsr, w_d[1, tt].rearrange("p (a b) -> p a b", a=NE))
                ps = sc_tile(f"psqk{tt}")
                for th in range(2):
                    mm3(ps[:, th * 512:(th + 1) * 512], wsl, wsr,
                        xnT8[:, :, th * 512:(th + 1) * 512],
                        xnTr[:, :, th * 512:(th + 1) * 512], NE)
                nc.vector.tensor_scalar(
                    out=dst, in0=ps, scalar1=RSW, scalar2=b_t[:, tt:tt + 1],
                    op0=ALU.mult, op1=ALU.add)
            return qtt, ktt

        def emit_scores(tt, qtt, ktt, e1s, zhs):
            """ST blocks + additive mask + one wide exp per (h, ki); even/odd
            heads interleaved so their K=64 matmuls share the PE array.
            Z row-sums are fused into the exp eviction via accum_out."""
            for h in (2 * tt, 2 * tt + 1):
                zhs[h] = zsp.tile([P, NT], F32, tag="zh", name=f"zh{h}")
            for ki in range(NT):
                for h in (2 * tt, 2 * tt + 1):
                    po = HD * (h % 2)
                    e1ki = e1p.tile([P, T], ATT_DT, tag="e1t", name=f"e1_{h}_{ki}")
                    e1s[(h, ki)] = e1ki
                    c0 = (ki * P) // 512
                    sps = sc_tile(f"pss{h}_{ki}")
                    for c in range(c0, 2):
                        nc.tensor.matmul(
                            sps[:, c * 512:(c + 1) * 512],
                            lhsT=ktt[po:po + HD, ki * P:(ki + 1) * P],
                            rhs=qtt[po:po + HD, c * 512:(c + 1) * 512],
                            start=True, stop=True)
                    nc.vector.tensor_add(
                        out=sps[:, ki * P:(ki + 1) * P],
                        in0=sps[:, ki * P:(ki + 1) * P], in1=maskd)
                    nc.scalar.activation(
                        out=e1ki[:, ki * P:], in_=sps[:, ki * P:],
                        func=AF.Exp, scale=SCALE,
                        accum_out=zhs[h][:, ki:ki + 1])

        def emit_av(hp, e1s, zhs):
            for h in (2 * hp, 2 * hp + 1):
                rz = zsp.tile([P, NT], F32, tag="rz")
                nc.vector.reciprocal(rz, zhs[h])
                vps = []
                for ki in range(NT):
                    vp_t = vpp.tile([P, HD], ATT_DT, tag="vp")
                    nc.vector.tensor_scalar_mul(
                        vp_t, vt[ki][:, h * HD:(h + 1) * HD], rz[:, ki:ki + 1])
                    vps.append(vp_t)
                po_ps = av_tile(f"psav{h}")
                for qi in range(NT):
                    for ki in range(qi + 1):
                        nc.tensor.matmul(
                            po_ps[:, qi * HD:(qi + 1) * HD],
                            lhsT=e1s[(h, ki)][:, qi * P:(qi + 1) * P],
                            rhs=vps[ki],
                            start=(ki == 0), stop=(ki == qi))
                nc.vector.tensor_copy(
                    out=concat_v[:, :, h, :],
                    in_=po_ps.rearrange("p (a d) -> p a d", a=NT))

        e1s_by_hp = {}
        zhs = {}
        for tt in range(H // 2):
            qtt, ktt = emit_qk(tt)
            e1s_by_hp[tt] = {}
            emit_scores(tt, qtt, ktt, e1s_by_hp[tt], zhs)
            if tt >= 1:
                emit_av(tt - 1, e1s_by_hp.pop(tt - 1), zhs)
        emit_av(H // 2 - 1, e1s_by_hp.pop(H // 2 - 1), zhs)

        zsp.release(); vpp.release(); e1p.release()
        wqk_pool.release(); qk_pool.release()
        vt_pool.release()
        xnT_pool.release()

        # =========== Phase 4: LN2 + transpose (fp8 pair) ===========
        anT_pool = tc.alloc_tile_pool(name="anT", bufs=1, side="right")
        anT8 = anT_pool.tile([P, NE, T], FP8, tag="anT8", name="anT8")
        anTr = anT_pool.tile([P, NE, T], FP8, tag="anTr", name="anTr")
        # prefetch the first FFN weight half while LN2 runs
        w1_tiles = {}
        for fl in range(NFH):
            w1q = w1s.tile([P, NE, P], FP8, tag="w1q", name=f"w1q{fl}")
            w1r = w1s.tile([P, NE, P], FP8, tag="w1r", name=f"w1r{fl}")
            nc.sync.dma_start(w1q, w1_d[0, fl].rearrange("p (a b) -> p a b", a=NE))
            nc.sync.dma_start(w1r, w1_d[1, fl].rearrange("p (a b) -> p a b", a=NE))
            w1_tiles[fl] = (w1q, w1r)
        with tc.tile_pool(name="ph4", bufs=3, side="left") as ph4:
            layernorm_to_transposed(
                lambda ti: concat[:, ti * E:(ti + 1) * E], anT8, anTr,
                g2_t, be2_t, ph4)
        concat_pool.release()

        # =========== Phase 5: FFN (two ff halves; weights fetched once) ===========
        out_pool = tc.alloc_tile_pool(name="outT", bufs=1, side="right")
        outT = [out_pool.tile([P, T], F32, tag=f"o{j}", name=f"o{j}")
                for j in range(NE)]
        with tc.tile_pool(name="w2s", bufs=6, side="left") as w2s, \
             tc.tile_pool(name="hid", bufs=2, side="left") as hidp:
            for ffh in range(2):
                hid8 = hidp.tile([P, NFH, T], FP8, tag="hid8", name=f"hid8_{ffh}", bufs=2)
                hidr = hidp.tile([P, NFH, T], FP8, tag="hidr", name=f"hidr_{ffh}", bufs=2)
                for fl in range(NFH):
                    fo = ffh * NFH + fl
                    if ffh == 1:
                        w1q = w1s.tile([P, NE, P], FP8, tag="w1q", name=f"w1q{fo}")
                        w1r = w1s.tile([P, NE, P], FP8, tag="w1r", name=f"w1r{fo}")
                        nc.sync.dma_start(
                            w1q, w1_d[0, fo].rearrange("p (a b) -> p a b", a=NE))
                        nc.sync.dma_start(
                            w1r, w1_d[1, fo].rearrange("p (a b) -> p a b", a=NE))
                    else:
                        w1q, w1r = w1_tiles[fl]
                    ps = sc_tile(f"psf{fo}")
                    for th in range(2):
                        mm3(ps[:, th * 512:(th + 1) * 512], w1q, w1r,
                            anT8[:, :, th * 512:(th + 1) * 512],
                            anTr[:, :, th * 512:(th + 1) * 512], NE)
                    # gelu once into f32, then split into fp8 main + residual
                    htmp = hidp.tile([P, T], F32, tag="htmp", bufs=3)
                    nc.scalar.activation(
                        out=htmp, in_=ps, func=AF.Gelu,
                        bias=b1_t[:, fo:fo + 1], scale=RSW)
                    nc.vector.tensor_copy(out=hid8[:, fl, :], in_=htmp)
                    nc.vector.tensor_tensor(
                        out=hidr[:, fl, :], in0=htmp, in1=hid8[:, fl, :],
                        op=ALU.subtract)
                for eo in range(NE):
                    w2q = w2s.tile([P, NFH, P], FP8, tag="w2q")
                    w2r = w2s.tile([P, NFH, P], FP8, tag="w2r")
                    nc.sync.dma_start(
                        w2q, w2_d[0, eo, :, ffh * NFH * P:(ffh + 1) * NFH * P]
                        .rearrange("p (a b) -> p a b", a=NFH))
                    nc.sync.dma_start(
                        w2r, w2_d[1, eo, :, ffh * NFH * P:(ffh + 1) * NFH * P]
                        .rearrange("p (a b) -> p a b", a=NFH))
                    ps = sc_tile(f"pso{ffh}_{eo}")
                    for th in range(2):
                        mm3(ps[:, th * 512:(th + 1) * 512], w2q, w2r,
                            hid8[:, :, th * 512:(th + 1) * 512],
                            hidr[:, :, th * 512:(th + 1) * 512], NFH)
                    if ffh == 0:
                        nc.scalar.activation(
                            out=outT[eo], in_=ps,
                            func=AF.Identity, bias=b2_t[:, eo:eo + 1], scale=RSW)
                    else:
                        otmp = w2s.tile([P, T], F32, tag="otmp", bufs=2)
                        nc.scalar.activation(
                            out=otmp, in_=ps, func=AF.Identity, scale=RSW)
                        nc.vector.tensor_add(out=outT[eo], in0=otmp, in1=outT[eo])
        w1s.release()
        for eo in range(NE):
            nc.sync.dma_start(y_d[eo * P:(eo + 1) * P, :], outT[eo])
        out_pool.release()
        anT_pool.release()
        if rep_cm is not None:
            rep_cm.__exit__(None, None, None)

    nc.compile()
    return nc


def _fp8_pair(w, scale=SW):
    """Split w*scale into (main, residual) e4m3 arrays."""
    import ml_dtypes
    ws = np.asarray(w, np.float32) * scale
    q = np.clip(ws, -240.0, 240.0).astype(ml_dtypes.float8_e4m3)
    r = (ws - q.astype(np.float32)).astype(ml_dtypes.float8_e4m3)
    return q, r


def host_inputs(core_x, Wq, bq, Wk, bk, Wv, bv, W1, b1, W2, b2, g1, be1, g2, be2):
    """Build the per-core in_map from full weights + this core's x shard [T, E]."""
    f = np.float32

    consts = np.zeros((P, _CONST_W), f)

    def put(name, arr):
        o, w = _CONST_COLS[name]
        consts[:, o:o + w] = arr

    put("bqt", np.asarray(bq, f).reshape(NE, P).T)
    put("bkt", np.asarray(bk, f).reshape(NE, P).T)
    put("b2t", np.asarray(b2, f).reshape(NE, P).T)
    put("g1t", np.asarray(g1, f).reshape(NE, P).T)
    put("be1t", np.asarray(be1, f).reshape(NE, P).T)
    put("g2t", np.asarray(g2, f).reshape(NE, P).T)
    put("be2t", np.asarray(be2, f).reshape(NE, P).T)
    put("b1t", np.asarray(b1, f).reshape(NF, P).T)
    put("ident", np.eye(P, dtype=f))
    put("maskd", np.where(np.triu(np.ones((P, P), bool)), 0.0, -6000.0).astype(f))
    put("bvb", np.broadcast_to(np.asarray(bv, f), (P, E)))

    def pack_proj(W, nout):
        """(E, nout*P) -> (nout, P, E) stationary layout, fp8 main+resid
        stacked as (2, nout, P, E)."""
        q, r = _fp8_pair(W)
        def lay(a):
            return np.ascontiguousarray(
                a.reshape(NE, P, nout, P).transpose(2, 1, 0, 3).reshape(nout, P, E))
        return np.stack([lay(q), lay(r)], axis=0)

    q, r = _fp8_pair(Wv)
    wv8 = np.stack([a.reshape(NE // 2, 2, P, E).transpose(0, 2, 1, 3)
                    for a in (q, r)], axis=0)

    return {
        "x": np.ascontiguousarray(core_x, f),
        "wq8": pack_proj(np.asarray(Wq, f), NE),
        "wk8": pack_proj(np.asarray(Wk, f), NE),
        "wv8": np.ascontiguousarray(wv8),
        "w18": pack_proj(np.asarray(W1, f), NF),
        "w28": np.stack([np.ascontiguousarray(
            a.reshape(NF, P, NE, P).transpose(2, 1, 0, 3).reshape(NE, P, FF))
            for a in _fp8_pair(W2)], axis=0),
        "consts": consts,
    }



# ======================================================================
# Harness entry point: full-input kernel with internal batch sharding
# ======================================================================

_NC_CACHE = {}


def _get_nc():
    key = ("bf16" if ATT_DT == BF16 else "f32")
    if key not in _NC_CACHE:
        _NC_CACHE[key] = build_decoder()
    return _NC_CACHE[key]


def kernel(x, Wq, bq, Wk, bk, Wv, bv, W1, b1, W2, b2, g1, be1, g2, be2):
    """Full-input entry: x [8, 1024, 1024]; returns [8, 1024, 1024] float32.

    Shards batch across the 8 NeuronCores (one element per core), runs the
    Bass decoder kernel SPMD, and gathers/transposes the per-core outputs.
    """
    from concourse.bass_utils import run_bass_kernel_spmd

    x = np.asarray(x, np.float32)
    B = x.shape[0]
    nc = _get_nc()
    args = tuple(np.asarray(a, np.float32) for a in
                 (Wq, bq, Wk, bk, Wv, bv, W1, b1, W2, b2, g1, be1, g2, be2))
    in_maps = [host_inputs(x[c], *args) for c in range(B)]
    res = run_bass_kernel_spmd(nc, in_maps, core_ids=list(range(B)))
    out = np.stack([np.asarray(r["yT"]).T for r in res.results], axis=0)
    return np.ascontiguousarray(out, np.float32)


# revision 7
# speedup vs baseline: 1.2988x; 1.1202x over previous
"""Bass/Tile kernel for one transformer decoder layer, data-parallel over batch.

Per-core work (one batch element): LN1 -> QKV -> causal attention with
query-axis softmax -> LN2 -> FFN(gelu).

Key math note: the reference softmaxes over the QUERY axis (axis=2), i.e. each
key-column k is normalized over queries q >= k. Therefore
  out[q,d] = sum_k exp(s[q,k]) * (V[k,d] / Z[k]),   Z[k] = sum_{q>=k} exp(s[q,k])
so the 1/Z folds into V's rows and no [T,T] division is needed.
We compute ST = S^T in [k, q] layout (ST = K @ Q^T / sqrt(E)) so that
Z is a free-axis reduction and the AV matmul needs no transposes.

All large GEMMs (Q/K/V projections and both FFN layers) run in fp8e4
DoubleRow mode with residual compensation: weights are host-split into
W ~ (W8 + R8)/SW with both parts e4m3; activations are split on-chip into
X8 + RX8 by quantize + subtract. The product is computed as
  X8@W8 + X8@R8 + RX8@W8   (the RX8@R8 term is ~eps^2 and dropped)
with all three terms accumulated in one PSUM group, so dequantization is a
single 1/SW at eviction. DoubleRow packs two 128-row K-tiles per matmul at
0.5 cycles/row, so the 3-term compensated GEMM costs 0.75x an fp32r GEMM
while keeping ~fp32 accuracy (residuals capture the quantization error).

Attention scores/AV stay in fp32r/bf16. Z row-sums are fused into the exp
activation via accum_out; the causal mask add runs on the (otherwise idle)
Pool engine.

Structure per core:
  x [T,E] --LN1--> xn --PE transpose--> xnT8/xnTr [E,T] fp8 pair
  V = xn @ Wv (token-major), then per head pair tt:
      QT/KT rows for pair tt (rotating slots)
      ST blocks -> additive causal mask on PSUM (Pool) -> one wide exp with
      fused Z accumulation -> E1T
      previous pair: V' = V * (1/Z), O += E1T^T @ V'
  LN2(concat) -> anT8/anTr;  FFN in two ff-halves (weights fetched once),
  gelu+bias fused, hid split to fp8 pair;  out written transposed [E,T].
"""

from contextlib import ExitStack

import numpy as np

import concourse.bass as bass
import concourse.tile as tile
import concourse.bacc as bacc
from concourse import mybir

F32 = mybir.dt.float32
F32R = mybir.dt.float32r
BF16 = mybir.dt.bfloat16
FP8 = mybir.dt.float8e4
AF = mybir.ActivationFunctionType
ALU = mybir.AluOpType
AX = mybir.AxisListType
DR = mybir.MatmulPerfMode.DoubleRow

P = 128
T = 1024
E = 1024
H = 16
HD = 64
FF = 4096
NT = T // P
NE = E // P
NF = FF // P
SCALE = 1.0 / 32.0  # 1/sqrt(E)
EPS = 1e-5
SW = 32.0  # host-side fp8 weight scale; divided out at PSUM eviction
RSW = 1.0 / SW

ATT_DT = BF16

# packed const layout (columns in the "consts" dram tensor)
_CONST_COLS = {
    "bqt": (0, NE), "bkt": (NE, NE), "b2t": (2 * NE, NE), "g1t": (3 * NE, NE),
    "be1t": (4 * NE, NE), "g2t": (5 * NE, NE), "be2t": (6 * NE, NE),
    "b1t": (7 * NE, NF), "ident": (7 * NE + NF, P), "maskd": (7 * NE + NF + P, P),
    "bvb": (7 * NE + NF + 2 * P, E),
}
_CONST_W = 7 * NE + NF + 2 * P + E


def build_decoder(debug=False, att_dt=None, reps=1):
    global ATT_DT
    if att_dt is not None:
        ATT_DT = att_dt
    nc = bacc.Bacc(None, target_bir_lowering=False, debug=debug)

    # ---------------- DRAM I/O ----------------
    x_d = nc.dram_tensor("x", (T, E), F32, kind="ExternalInput")
    wq_d = nc.dram_tensor("wq8", (2, NE, P, E), FP8, kind="ExternalInput")
    wk_d = nc.dram_tensor("wk8", (2, NE, P, E), FP8, kind="ExternalInput")
    wv_d = nc.dram_tensor("wv8", (2, NE // 2, P, 2, E), FP8, kind="ExternalInput")
    w1_d = nc.dram_tensor("w18", (2, NF, P, E), FP8, kind="ExternalInput")
    w2_d = nc.dram_tensor("w28", (2, NE, P, FF), FP8, kind="ExternalInput")
    c_d = nc.dram_tensor("consts", (P, _CONST_W), F32, kind="ExternalInput")
    y_d = nc.dram_tensor("yT", (E, T), F32, kind="ExternalOutput")

    with ExitStack() as es:
        tc = es.enter_context(tile.TileContext(nc))

        const = es.enter_context(tc.tile_pool(name="const", bufs=1, side="left"))
        # one PSUM pool, 8 banks total: 3x [P,1024] (proj/scores/FFN) +
        # 2x [P,512] shared by AV accumulation and LN transposes (never overlap)
        psq = es.enter_context(tc.tile_pool(name="psq", bufs=1, space="PSUM"))

        cz = const.tile([P, _CONST_W], F32)
        nc.sync.dma_start(cz, c_d[:, :])

        def cview(name):
            o, w = _CONST_COLS[name]
            return cz[:, o:o + w]

        bq_t, bk_t, b2_t = cview("bqt"), cview("bkt"), cview("b2t")
        g1_t, be1_t, g2_t, be2_t = (cview("g1t"), cview("be1t"),
                                    cview("g2t"), cview("be2t"))
        b1_t, ident, maskd, bv_b = (cview("b1t"), cview("ident"),
                                    cview("maskd"), cview("bvb"))
        eps_t = const.tile([P, 1], F32)
        nc.vector.memset(eps_t, EPS)
        ident_r = const.tile([P, P], F32R)
        nc.vector.tensor_copy(ident_r, ident)

        def sc_tile(name):
            return psq.tile([P, 2 * 512], F32, tag="sc", bufs=3, name=name)

        def av_tile(name, dt=F32):
            return psq.tile([P, 512], dt, tag="avtr", bufs=2, name=name)

        def layernorm_to_transposed(src_fn, dst_q, dst_r, g_t, b_t, ln_pool):
            """src_fn(ti) -> [P, E] token-major AP.  Writes the fp8 pair
            dst_q/dst_r [P, NE, T] feature-major: q = fp8(norm^T*g+b),
            r = fp8(full - q)."""
            for ti in range(NT):
                xsrc = src_fn(ti)
                stats = ln_pool.tile([P, 2, 6], F32, tag="bnstats")
                for sg in range(2):
                    nc.vector.bn_stats(out=stats[:, sg, :],
                                       in_=xsrc[:, sg * 512:(sg + 1) * 512])
                mv = ln_pool.tile([P, 2], F32, tag="bnmv")
                nc.vector.bn_aggr(out=mv, in_=stats)
                nc.scalar.activation(out=mv[:, 1:2], in_=mv[:, 1:2], func=AF.Sqrt,
                                     bias=eps_t)
                nc.vector.reciprocal(mv[:, 1:2], mv[:, 1:2])
                xn = ln_pool.tile([P, E], F32R, tag="xn")
                nc.vector.tensor_scalar(
                    out=xn, in0=xsrc, scalar1=mv[:, 0:1], scalar2=mv[:, 1:2],
                    op0=ALU.subtract, op1=ALU.mult)
                for jb in range(NE // 4):
                    ptr = av_tile(f"tr{ti}_{jb}", dt=F32R)
                    tmp = ln_pool.tile([P, 4 * P], F32, tag="lntmp")
                    for jj in range(4):
                        j = jb * 4 + jj
                        nc.tensor.transpose(ptr[:, jj * P:(jj + 1) * P],
                                            xn[:, j * P:(j + 1) * P], ident_r)
                        # per-feature g,b applied post-transpose (features are
                        # partitions here, so g/b are per-partition scalars)
                        nc.scalar.activation(
                            out=tmp[:, jj * P:(jj + 1) * P],
                            in_=ptr[:, jj * P:(jj + 1) * P],
                            func=AF.Identity, bias=b_t[:, j:j + 1],
                            scale=g_t[:, j:j + 1])
                    j0 = jb * 4
                    qv = dst_q[:, j0:j0 + 4, ti * P:(ti + 1) * P]
                    tmpv = tmp.rearrange("p (a b) -> p a b", a=4)
                    nc.vector.tensor_copy(out=qv, in_=tmpv)
                    nc.vector.tensor_tensor(
                        out=dst_r[:, j0:j0 + 4, ti * P:(ti + 1) * P],
                        in0=tmpv, in1=qv, op=ALU.subtract)

        def mm3(ps_slice, wq_tile, wr_tile, xq_ap, xr_ap, nk):
            """3-term compensated fp8 DoubleRow GEMM into one PSUM group.
            wq_tile/wr_tile: [P, nk, M] fp8; xq_ap/xr_ap: [P, nk, N] fp8."""
            terms = ((wq_tile, xq_ap), (wr_tile, xq_ap), (wq_tile, xr_ap))
            nkh = nk // 2
            n_inst = 3 * nkh
            i = 0
            for wt, xt in terms:
                for kp in range(nkh):
                    nc.tensor.matmul(
                        ps_slice,
                        lhsT=wt[:, 2 * kp:2 * kp + 2, :],
                        rhs=xt[:, 2 * kp:2 * kp + 2, :],
                        start=(i == 0), stop=(i == n_inst - 1), perf_mode=DR)
                    i += 1

        rep_cm = tc.For_i(0, reps, 1) if reps > 1 else None
        if rep_cm is not None:
            rep_cm.__enter__()

        # =========== Phase 1: LN1 + transpose (fp8 pair) ===========
        xnT_pool = tc.alloc_tile_pool(name="xnT", bufs=1, side="right")
        xnT8 = xnT_pool.tile([P, NE, T], FP8, tag="xnT8", name="xnT8")
        xnTr = xnT_pool.tile([P, NE, T], FP8, tag="xnTr", name="xnTr")
        with tc.tile_pool(name="ph1", bufs=3, side="left") as ph1, \
             tc.tile_pool(name="xin", bufs=NT, side="left") as xin:
            x_tiles = []
            for ti in range(NT):
                xt = xin.tile([P, E], F32, tag="x")
                nc.sync.dma_start(xt, x_d[ti * P:(ti + 1) * P, :])
                x_tiles.append(xt)
            layernorm_to_transposed(lambda ti: x_tiles[ti][:, :], xnT8, xnTr,
                                    g1_t, be1_t, ph1)

        # ====== Phase 2+3: V, then per-pair {Q/K proj -> scores+exp -> AV} ======
        # w1s is created early so it sits below concat on the left pool stack
        # (LIFO release order); its DMAs are issued at the start of LN2.
        NFH = NF // 2
        w1s = tc.alloc_tile_pool(name="w1s", bufs=NFH + 1, side="left")
        concat_pool = tc.alloc_tile_pool(name="concat", bufs=1, side="left")
        concat = concat_pool.tile([P, NT * E], ATT_DT)
        concat_v = concat.rearrange("p (a h d) -> p a h d", a=NT, h=H)

        vt_pool = tc.alloc_tile_pool(name="vtp", bufs=1, side="left")
        vt = [vt_pool.tile([P, E], ATT_DT, tag=f"vt{i}", name=f"vt{i}")
              for i in range(NT)]

        # --- V projection (token-major): xn stationary, Wv moving ---
        with tc.tile_pool(name="wv", bufs=NE + 2, side="left") as wvp:
            wtl = []
            for m in range(2):  # main, residual
                for ko in range(NE // 2):
                    wtile = wvp.tile([P, 2, E], FP8, tag="w", name=f"wv{m}_{ko}")
                    nc.sync.dma_start(wtile, wv_d[m, ko])
                    wtl.append(wtile)
            for ti in range(NT):
                ps = sc_tile(f"psv{ti}")
                for nh in range(2):
                    pss = ps[:, nh * 512:(nh + 1) * 512]
                    i = 0
                    for xs, woff in ((xnT8, 0), (xnT8, NE // 2), (xnTr, 0)):
                        for kp in range(NE // 2):
                            nc.tensor.matmul(
                                pss,
                                lhsT=xs[:, 2 * kp:2 * kp + 2, ti * P:(ti + 1) * P],
                                rhs=wtl[woff + kp][:, :, nh * 512:(nh + 1) * 512],
                                start=(i == 0), stop=(i == 3 * (NE // 2) - 1),
                                perf_mode=DR)
                            i += 1
                vtmp = wvp.tile([P, E], F32, tag="vtmp", bufs=2)
                nc.vector.tensor_scalar(
                    out=vtmp, in0=ps, scalar1=RSW, scalar2=None, op0=ALU.mult)
                nc.vector.tensor_add(out=vt[ti], in0=vtmp, in1=bv_b)

        # --- interleaved Q/K projection + attention ---
        qk_pool = tc.alloc_tile_pool(name="qk", bufs=2, side="left")
        wqk_pool = tc.alloc_tile_pool(name="wqk", bufs=6, side="left")
        e1p = tc.alloc_tile_pool(name="e1", bufs=32, side="left")
        vpp = tc.alloc_tile_pool(name="vp", bufs=16, side="left")
        zsp = tc.alloc_tile_pool(name="zs", bufs=8, side="left")

        def emit_qk(tt):
            qtt = qk_pool.tile([P, T], F32R, tag="qt", name=f"qt{tt}")
            ktt = qk_pool.tile([P, T], F32R, tag="kt", name=f"kt{tt}")
            for dst, w_d, b_t in ((qtt, wq_d, bq_t), (ktt, wk_d, bk_t)):
                wsl = wqk_pool.tile([P, NE, P], FP8, tag="wqk")
                wsr = wqk_pool.tile([P, NE, P], FP8, tag="wqkr")
                nc.sync.dma_start(wsl, w_d[0, tt].rearrange("p (a b) -> p a b", a=NE))
                nc.sync.dma_start(wsr, w_d[1, tt].rearrange("p (a b) -> p a b", a=NE))
                for th in range(2):
                    ps = av_tile(f"psqk{tt}_{th}")
                    mm3(ps, wsl, wsr,
                        xnT8[:, :, th * 512:(th + 1) * 512],
                        xnTr[:, :, th * 512:(th + 1) * 512], NE)
                    nc.vector.tensor_scalar(
                        out=dst[:, th * 512:(th + 1) * 512], in0=ps,
                        scalar1=RSW, scalar2=b_t[:, tt:tt + 1],
                        op0=ALU.mult, op1=ALU.add)
            return qtt, ktt

        def emit_scores(tt, qtt, ktt, e1s, zhs):
            """ST blocks + additive mask + one wide exp per (h, ki); even/odd
            heads interleaved so their K=64 matmuls share the PE array.
            Z row-sums are fused into the exp eviction via accum_out."""
            for h in (2 * tt, 2 * tt + 1):
                zhs[h] = zsp.tile([P, NT], F32, tag="zh", name=f"zh{h}")
            for ki in range(NT):
                for h in (2 * tt, 2 * tt + 1):
                    po = HD * (h % 2)
                    e1ki = e1p.tile([P, T], ATT_DT, tag="e1t", name=f"e1_{h}_{ki}")
                    e1s[(h, ki)] = e1ki
                    c0 = (ki * P) // 512
                    sps = sc_tile(f"pss{h}_{ki}")
                    for c in range(c0, 2):
                        nc.tensor.matmul(
                            sps[:, c * 512:(c + 1) * 512],
                            lhsT=ktt[po:po + HD, ki * P:(ki + 1) * P],
                            rhs=qtt[po:po + HD, c * 512:(c + 1) * 512],
                            start=True, stop=True)
                    nc.vector.tensor_add(
                        out=sps[:, ki * P:(ki + 1) * P],
                        in0=sps[:, ki * P:(ki + 1) * P], in1=maskd)
                    nc.scalar.activation(
                        out=e1ki[:, ki * P:], in_=sps[:, ki * P:],
                        func=AF.Exp, scale=SCALE,
                        accum_out=zhs[h][:, ki:ki + 1])

        def emit_av(hp, e1s, zhs):
            for h in (2 * hp, 2 * hp + 1):
                rz = zsp.tile([P, NT], F32, tag="rz")
                nc.vector.reciprocal(rz, zhs[h])
                vps = []
                for ki in range(NT):
                    vp_t = vpp.tile([P, HD], ATT_DT, tag="vp")
                    nc.vector.tensor_scalar_mul(
                        vp_t, vt[ki][:, h * HD:(h + 1) * HD], rz[:, ki:ki + 1])
                    vps.append(vp_t)
                po_ps = av_tile(f"psav{h}")
                for qi in range(NT):
                    for ki in range(qi + 1):
                        nc.tensor.matmul(
                            po_ps[:, qi * HD:(qi + 1) * HD],
                            lhsT=e1s[(h, ki)][:, qi * P:(qi + 1) * P],
                            rhs=vps[ki],
                            start=(ki == 0), stop=(ki == qi))
                nc.vector.tensor_copy(
                    out=concat_v[:, :, h, :],
                    in_=po_ps.rearrange("p (a d) -> p a d", a=NT))

        e1s_by_hp = {}
        zhs = {}
        for tt in range(H // 2):
            qtt, ktt = emit_qk(tt)
            e1s_by_hp[tt] = {}
            emit_scores(tt, qtt, ktt, e1s_by_hp[tt], zhs)
            if tt >= 1:
                emit_av(tt - 1, e1s_by_hp.pop(tt - 1), zhs)
        emit_av(H // 2 - 1, e1s_by_hp.pop(H // 2 - 1), zhs)

        zsp.release(); vpp.release(); e1p.release()
        wqk_pool.release(); qk_pool.release()
        vt_pool.release()
        xnT_pool.release()

        # =========== Phase 4: LN2 + transpose (fp8 pair) ===========
        anT_pool = tc.alloc_tile_pool(name="anT", bufs=1, side="right")
        anT8 = anT_pool.tile([P, NE, T], FP8, tag="anT8", name="anT8")
        anTr = anT_pool.tile([P, NE, T], FP8, tag="anTr", name="anTr")
        # prefetch the first FFN weight half while LN2 runs
        w1_tiles = {}
        for fl in range(NFH):
            w1q = w1s.tile([P, NE, P], FP8, tag="w1q", name=f"w1q{fl}")
            w1r = w1s.tile([P, NE, P], FP8, tag="w1r", name=f"w1r{fl}")
            nc.sync.dma_start(w1q, w1_d[0, fl].rearrange("p (a b) -> p a b", a=NE))
            nc.sync.dma_start(w1r, w1_d[1, fl].rearrange("p (a b) -> p a b", a=NE))
            w1_tiles[fl] = (w1q, w1r)
        with tc.tile_pool(name="ph4", bufs=3, side="left") as ph4:
            layernorm_to_transposed(
                lambda ti: concat[:, ti * E:(ti + 1) * E], anT8, anTr,
                g2_t, be2_t, ph4)
        concat_pool.release()

        # =========== Phase 5: FFN (two ff halves; weights fetched once) ===========
        out_pool = tc.alloc_tile_pool(name="outT", bufs=1, side="right")
        outT = [out_pool.tile([P, T], F32, tag=f"o{j}", name=f"o{j}")
                for j in range(NE)]
        with tc.tile_pool(name="w2s", bufs=6, side="left") as w2s, \
             tc.tile_pool(name="hid", bufs=2, side="left") as hidp:
            for ffh in range(2):
                hid8 = hidp.tile([P, NFH, T], FP8, tag="hid8", name=f"hid8_{ffh}", bufs=2)
                hidr = hidp.tile([P, NFH, T], FP8, tag="hidr", name=f"hidr_{ffh}", bufs=2)
                for fl in range(NFH):
                    fo = ffh * NFH + fl
                    if ffh == 1:
                        w1q = w1s.tile([P, NE, P], FP8, tag="w1q", name=f"w1q{fo}")
                        w1r = w1s.tile([P, NE, P], FP8, tag="w1r", name=f"w1r{fo}")
                        nc.sync.dma_start(
                            w1q, w1_d[0, fo].rearrange("p (a b) -> p a b", a=NE))
                        nc.sync.dma_start(
                            w1r, w1_d[1, fo].rearrange("p (a b) -> p a b", a=NE))
                    else:
                        w1q, w1r = w1_tiles[fl]
                    ps = sc_tile(f"psf{fo}")
                    for th in range(2):
                        mm3(ps[:, th * 512:(th + 1) * 512], w1q, w1r,
                            anT8[:, :, th * 512:(th + 1) * 512],
                            anTr[:, :, th * 512:(th + 1) * 512], NE)
                    # gelu once into f32, then split into fp8 main + residual
                    htmp = hidp.tile([P, T], F32, tag="htmp", bufs=3)
                    nc.scalar.activation(
                        out=htmp, in_=ps, func=AF.Gelu,
                        bias=b1_t[:, fo:fo + 1], scale=RSW)
                    nc.vector.tensor_copy(out=hid8[:, fl, :], in_=htmp)
                    nc.vector.tensor_tensor(
                        out=hidr[:, fl, :], in0=htmp, in1=hid8[:, fl, :],
                        op=ALU.subtract)
                for eo in range(NE):
                    w2q = w2s.tile([P, NFH, P], FP8, tag="w2q")
                    w2r = w2s.tile([P, NFH, P], FP8, tag="w2r")
                    nc.sync.dma_start(
                        w2q, w2_d[0, eo, :, ffh * NFH * P:(ffh + 1) * NFH * P]
                        .rearrange("p (a b) -> p a b", a=NFH))
                    nc.sync.dma_start(
                        w2r, w2_d[1, eo, :, ffh * NFH * P:(ffh + 1) * NFH * P]
                        .rearrange("p (a b) -> p a b", a=NFH))
                    ps = sc_tile(f"pso{ffh}_{eo}")
                    for th in range(2):
                        mm3(ps[:, th * 512:(th + 1) * 512], w2q, w2r,
                            hid8[:, :, th * 512:(th + 1) * 512],
                            hidr[:, :, th * 512:(th + 1) * 512], NFH)
                    if ffh == 0:
                        nc.scalar.activation(
                            out=outT[eo], in_=ps,
                            func=AF.Identity, bias=b2_t[:, eo:eo + 1], scale=RSW)
                    else:
                        otmp = w2s.tile([P, T], F32, tag="otmp", bufs=2)
                        nc.scalar.activation(
                            out=otmp, in_=ps, func=AF.Identity, scale=RSW)
                        nc.vector.tensor_add(out=outT[eo], in0=otmp, in1=outT[eo])
        w1s.release()
        for eo in range(NE):
            nc.sync.dma_start(y_d[eo * P:(eo + 1) * P, :], outT[eo])
        out_pool.release()
        anT_pool.release()
        if rep_cm is not None:
            rep_cm.__exit__(None, None, None)

    nc.compile()
    return nc


def _fp8_pair(w, scale=SW):
    """Split w*scale into (main, residual) e4m3 arrays."""
    import ml_dtypes
    ws = np.asarray(w, np.float32) * scale
    q = np.clip(ws, -240.0, 240.0).astype(ml_dtypes.float8_e4m3)
    r = (ws - q.astype(np.float32)).astype(ml_dtypes.float8_e4m3)
    return q, r


def host_inputs(core_x, Wq, bq, Wk, bk, Wv, bv, W1, b1, W2, b2, g1, be1, g2, be2):
    """Build the per-core in_map from full weights + this core's x shard [T, E]."""
    f = np.float32

    consts = np.zeros((P, _CONST_W), f)

    def put(name, arr):
        o, w = _CONST_COLS[name]
        consts[:, o:o + w] = arr

    put("bqt", np.asarray(bq, f).reshape(NE, P).T)
    put("bkt", np.asarray(bk, f).reshape(NE, P).T)
    put("b2t", np.asarray(b2, f).reshape(NE, P).T)
    put("g1t", np.asarray(g1, f).reshape(NE, P).T)
    put("be1t", np.asarray(be1, f).reshape(NE, P).T)
    put("g2t", np.asarray(g2, f).reshape(NE, P).T)
    put("be2t", np.asarray(be2, f).reshape(NE, P).T)
    put("b1t", np.asarray(b1, f).reshape(NF, P).T)
    put("ident", np.eye(P, dtype=f))
    put("maskd", np.where(np.triu(np.ones((P, P), bool)), 0.0, -6000.0).astype(f))
    put("bvb", np.broadcast_to(np.asarray(bv, f), (P, E)))

    def pack_proj(W, nout):
        """(E, nout*P) -> (nout, P, E) stationary layout, fp8 main+resid
        stacked as (2, nout, P, E)."""
        q, r = _fp8_pair(W)
        def lay(a):
            return np.ascontiguousarray(
                a.reshape(NE, P, nout, P).transpose(2, 1, 0, 3).reshape(nout, P, E))
        return np.stack([lay(q), lay(r)], axis=0)

    q, r = _fp8_pair(Wv)
    wv8 = np.stack([a.reshape(NE // 2, 2, P, E).transpose(0, 2, 1, 3)
                    for a in (q, r)], axis=0)

    return {
        "x": np.ascontiguousarray(core_x, f),
        "wq8": pack_proj(np.asarray(Wq, f), NE),
        "wk8": pack_proj(np.asarray(Wk, f), NE),
        "wv8": np.ascontiguousarray(wv8),
        "w18": pack_proj(np.asarray(W1, f), NF),
        "w28": np.stack([np.ascontiguousarray(
            a.reshape(NF, P, NE, P).transpose(2, 1, 0, 3).reshape(NE, P, FF))
            for a in _fp8_pair(W2)], axis=0),
        "consts": consts,
    }



# ======================================================================
# Harness entry point: full-input kernel with internal batch sharding
# ======================================================================

_NC_CACHE = {}


def _get_nc():
    key = ("bf16" if ATT_DT == BF16 else "f32")
    if key not in _NC_CACHE:
        _NC_CACHE[key] = build_decoder()
    return _NC_CACHE[key]


def kernel(x, Wq, bq, Wk, bk, Wv, bv, W1, b1, W2, b2, g1, be1, g2, be2):
    """Full-input entry: x [8, 1024, 1024]; returns [8, 1024, 1024] float32.

    Shards batch across the 8 NeuronCores (one element per core), runs the
    Bass decoder kernel SPMD, and gathers/transposes the per-core outputs.
    """
    from concourse.bass_utils import run_bass_kernel_spmd

    x = np.asarray(x, np.float32)
    B = x.shape[0]
    nc = _get_nc()
    args = tuple(np.asarray(a, np.float32) for a in
                 (Wq, bq, Wk, bk, Wv, bv, W1, b1, W2, b2, g1, be1, g2, be2))
    in_maps = [host_inputs(x[c], *args) for c in range(B)]
    res = run_bass_kernel_spmd(nc, in_maps, core_ids=list(range(B)))
    out = np.stack([np.asarray(r["yT"]).T for r in res.results], axis=0)
    return np.ascontiguousarray(out, np.float32)
